# revision 65
# baseline (speedup 1.0000x reference)
"""Trainium2 Bass kernel for nn_MLP_Route_RL_Model (route RL model).

Reference math (per batch element b of 256):
  - state = [route_nums (48) | customers (48*24*36)]
  - customer MLP (tanh-tanh, 36->128->32) on every node of every route
  - 2-layer GRU (hidden 128) over the 24 nodes of each of the 48 routes
  - route summary mean, node-selection MLP 256->256->128->24, masked softmax

Sharding: pure data parallel over batch B=256 -> 8 cores x 32.

Schedule: the two GRU layers are software-pipelined with layer 1 lagging
layer 0 by one step (h1 double-buffered), so the 6 cells emitted per step
(3 chunks x 2 layers) are mutually independent.  Each cell is split into a
"front" (matmuls + merged r|z sigmoid + n-gate input prep) and a "back"
(tanh + hidden update), with back(k-2) emitted after front(k) so the
Activation engine never waits on the DVE chain of the cell it just gated.
The r and z gates share one 2-bank PSUM tile and one 1024-wide sigmoid;
their biases are pre-added by K=1 PE matmuls (bias row x ones).
Customer-MLP groups are interleaved one sub-block per step as filler, with
node pairs sharing one 1024-wide tanh.
"""

import contextlib
import os as _os
PDEPTH = int(_os.environ.get("PDEPTH", "2"))
import sys

import numpy as np

sys.path.insert(0, "/opt/trn_rl_repo")

import concourse.bass as bass  # noqa: E402
import concourse.bacc as bacc  # noqa: E402
import concourse.mybir as mybir  # noqa: E402
import concourse.tile as tile  # noqa: E402
from concourse.bass_utils import run_bass_kernel_spmd  # noqa: E402

F32 = mybir.dt.float32
F16 = mybir.dt.float16
AF = mybir.ActivationFunctionType
OP = mybir.AluOpType

# Problem shape constants
B = 256
NCORES = 8
BLOC = B // NCORES          # 32 batch rows per core
MR = 48                     # routes per batch
MN = 24                     # nodes per route
FEAT = 36
CH = 128                    # customer hidden
CO = 32                     # customer out
GH = 128                    # GRU hidden
S = BLOC * MR               # sequences per core = 1536
NC = 512                    # token chunk (PSUM bank width in fp32)
NCH = S // NC               # chunks per core = 3
NG = MN // 4                # node groups of 4 (cust_out partition stacking)
NQ = S // 128               # 128-token output groups = 12

_GBCOLS = [(l, g) for l in (0, 1) for g in ("r", "z", "zn", "in", "hn")]

_cache = {}


@contextlib.contextmanager
def _noprio():
    yield


def _build(reps=1):
    """Trace + schedule the per-core Tile kernel. Returns the Bass module."""
    nc = bacc.Bacc("TRN2", target_bir_lowering=False, debug=False)

    # ---- DRAM I/O ----------------------------------------------------------
    d_cust = nc.dram_tensor("cust_fm", [FEAT, MN * S], F16, kind="ExternalInput")
    d_wc1 = nc.dram_tensor("Wc1h", [FEAT, CH], F16, kind="ExternalInput")
    d_bc1 = nc.dram_tensor("bc1", [CH, 1], F32, kind="ExternalInput")
    d_wc2 = nc.dram_tensor("Wc2h", [CH, CO], F16, kind="ExternalInput")
    d_bc2 = nc.dram_tensor("bc2s", [128, 1], F32, kind="ExternalInput")
    d_wih0 = nc.dram_tensor("Wih0h", [128, 3 * GH], F16, kind="ExternalInput")
    d_whh0 = nc.dram_tensor("Whh0h", [GH, 3 * GH], F16, kind="ExternalInput")
    d_wih1 = nc.dram_tensor("Wih1h", [GH, 3 * GH], F16, kind="ExternalInput")
    d_whh1 = nc.dram_tensor("Whh1h", [GH, 3 * GH], F16, kind="ExternalInput")
    # all 10 GRU gate bias columns in one tensor (single DMA)
    d_gball = nc.dram_tensor("gball", [GH, 10], F32, kind="ExternalInput")
    # r|z bias rows for the PE pre-add: [b0_r|b0_z|b1_r|b1_z]
    d_brz = nc.dram_tensor("brz", [1, 4 * GH], F16, kind="ExternalInput")
    d_onesm = nc.dram_tensor("onesm", [1, NC], F16, kind="ExternalInput")
    d_wn1a = nc.dram_tensor("Wn1a", [GH, 256], F16, kind="ExternalInput")
    d_wn1b = nc.dram_tensor("Wn1b", [GH, 256], F16, kind="ExternalInput")
    d_bn1r = nc.dram_tensor("bn1row", [1, 256], F16, kind="ExternalInput")
    d_wn2a = nc.dram_tensor("Wn2a", [128, 128], F16, kind="ExternalInput")
    d_wn2b = nc.dram_tensor("Wn2b", [128, 128], F16, kind="ExternalInput")
    d_bn2 = nc.dram_tensor("bn2c", [128, 1], F32, kind="ExternalInput")
    d_wn3 = nc.dram_tensor("Wn3h", [GH, MN], F16, kind="ExternalInput")
    d_bn3 = nc.dram_tensor("bn3r", [1, MN], F32, kind="ExternalInput")
    d_sel = nc.dram_tensor("sel", [BLOC, S], F16, kind="ExternalInput")
    # iota (24 cols) | per-q-group route_nums (12 cols)
    d_mrn = nc.dram_tensor("mrn", [128, MN + NQ], F32, kind="ExternalInput")
    d_ident = nc.dram_tensor("ident128", [128, 128], F16, kind="ExternalInput")
    d_ones = nc.dram_tensor("ones128", [1, 128], F32, kind="ExternalInput")
    d_out = nc.dram_tensor("out_tm", [128, NQ * MN], F32, kind="ExternalOutput")

    with tile.TileContext(nc) as tc:
        with (
            tc.tile_pool(name="wpool", bufs=1) as wp,
            tc.tile_pool(name="state", bufs=1) as sp,
            tc.tile_pool(name="xin", bufs=10) as xp,
            tc.tile_pool(name="h1c", bufs=4) as h1p,
            tc.tile_pool(name="gates", bufs=8) as gp,
            tc.tile_pool(name="upd", bufs=8) as up,
            tc.tile_pool(name="fin", bufs=4) as fp_,
            tc.tile_pool(name="ps", bufs=2, space="PSUM") as ps,
        ):
            def wtile(dram, shape, dtype):
                t = wp.tile(shape, dtype, tag=dram.name)
                nc.sync.dma_start(t[:], dram.ap())
                return t

            # ---- phase-A-critical weights first (unblocks first tanh) ------
            wc1 = wtile(d_wc1, [FEAT, CH], F16)
            bc1 = wtile(d_bc1, [CH, 1], F32)
            wc2 = wtile(d_wc2, [CH, CO], F16)
            bc2 = wtile(d_bc2, [128, 1], F32)

            xtiles = {}

            def load_xc(n, eng=None):
                if n not in xtiles:
                    xn = xp.tile([FEAT, S], F16, tag="xc", name=f"xc{n}")
                    (eng or nc.sync).dma_start(
                        xn[:], d_cust.ap()[:, n * S : (n + 1) * S]
                    )
                    xtiles[n] = xn
                return xtiles[n]

            # first group's customer tiles go over SWDGE (Pool queue) in
            # parallel with the weight DMAs on the HWDGE path
            for n in range(4):
                load_xc(n, eng=nc.gpsimd)

            # warm the ACT function tables during the DMA-wait dead time at
            # the head.  tanh+sigmoid+relu share one table set, so this
            # costs a single load; exp is NOT warmed here (its set lacks
            # sigmoid and would force a reload before the GRU) — a dummy
            # exp after the GRU drain loads the exp set (which also covers
            # tanh+relu) during the idle mean-reduce window.
            warm = wp.tile([1, 1], F32, tag="warm")
            warm2 = wp.tile([1, 1], F32, tag="warm2")
            nc.vector.memset(warm[:], 0.0)
            for fn in (AF.Tanh, AF.Sigmoid, AF.Relu):
                nc.scalar.activation(warm2[:], warm[:], fn)

            # persistent state
            cust = sp.tile([128, NG * S], F16, tag="cust_out")
            h1a = sp.tile([GH, S], F16, tag="h1a")
            h1b = sp.tile([GH, S], F16, tag="h1b")
            h2 = sp.tile([GH, S], F16, tag="h2")
            stage = sp.tile([128, NQ * MN], F32, tag="stage")
            mskall = sp.tile([128, NQ * MN], F32, tag="mskall")

            # ---- layer-0 GRU weights (needed by step 0) --------------------
            wih0 = wtile(d_wih0, [128, 3 * GH], F16)
            gball = wtile(d_gball, [GH, 10], F32)
            brz = wtile(d_brz, [1, 4 * GH], F16)
            onesm = wtile(d_onesm, [1, NC], F16)
            ident = wtile(d_ident, [128, 128], F16)
            gb = {key: gball[:, i : i + 1] for i, key in enumerate(_GBCOLS)}

            W = {}  # late-loaded weights (filled during the step loop)

          # timing-calibration repeat loop (reps=1 in production)
          # fmt: off
            for _rep in range(reps):
              # ---- phase A: customer MLP (one (group, sub-block) slice) ----
              def emitA(g, sb):
                  c2 = ps.tile([128, NC], F32, tag="pi")
                  for kp in (0, 1):
                      pp = ps.tile([CH, 2 * NC], F32, tag="prz")
                      for i in (0, 1):
                          n = 4 * g + 2 * kp + i
                          nc.tensor.matmul(
                              pp[:, i * NC : (i + 1) * NC], wc1[:],
                              xtiles[n][:, sb * NC : (sb + 1) * NC],
                          )
                      hp2 = h1p.tile([CH, 2 * NC], F16, tag="h1c")
                      nc.scalar.activation(hp2[:], pp[:], AF.Tanh, bias=bc1[:])
                      for i in (0, 1):
                          k = 2 * kp + i
                          nc.tensor.matmul(
                              c2[32 * k : 32 * (k + 1), :], wc2[:],
                              hp2[:, i * NC : (i + 1) * NC],
                              tile_position=(0, 32 * k),
                          )
                  nc.scalar.activation(
                      cust[:, g * S + sb * NC : g * S + (sb + 1) * NC],
                      c2[:], AF.Tanh, bias=bc2[:],
                  )

              # ---- GRU cell, split into front (gates) and back (update) ----
              def cell_front(t, layer, c):
                  c0, c1 = c * NC, (c + 1) * NC
                  if layer == 0:
                      g, k = t // 4, t % 4
                      xap = cust[32 * k : 32 * (k + 1), g * S + c0 : g * S + c1]
                      kq = k
                      wih, whh = wih0, W.get("whh0")
                      hread = h1b if t % 2 == 0 else h1a
                      hwrite = h1a if t % 2 == 0 else h1b
                  else:
                      xap = (h1a if t % 2 == 0 else h1b)[:, c0:c1]
                      kq = None
                      wih, whh = W.get("wih1"), W.get("whh1")
                      hread = hwrite = h2
                  h = hread[:, c0:c1]
                  hp = tc.high_priority if layer == 0 else _noprio
                  bo = 2 * GH * layer
                  prz = ps.tile([GH, 2 * NC], F32, tag="prz")
                  pi = ps.tile([GH, NC], F32, tag="pi")
                  if t == 0:
                      # h == 0: whh terms vanish; separate sigmoids with the
                      # ACT bias operand (1-z via sigmoid(-x)).
                      if kq is not None:
                          p0 = 32 * kq
                          tp = (p0, 0)
                          wk = wih[p0 : p0 + CO, :]
                          nc.tensor.matmul(prz[:, 0:NC], wk[:, 0:GH], xap,
                                           tile_position=tp)
                          nc.tensor.matmul(prz[:, NC:], wk[:, GH : 2 * GH], xap,
                                           tile_position=tp)
                          nc.tensor.matmul(pi[:], wk[:, 2 * GH : 3 * GH], xap,
                                           start=True, stop=False,
                                           tile_position=tp)
                      else:
                          nc.tensor.matmul(prz[:, 0:NC], wih[:, 0:GH], xap)
                          nc.tensor.matmul(prz[:, NC:], wih[:, GH : 2 * GH], xap)
                          nc.tensor.matmul(pi[:], wih[:, 2 * GH : 3 * GH], xap,
                                           start=True, stop=False)
                      r = gp.tile([GH, NC], F16, tag="r")
                      nc.scalar.activation(r[:], prz[:, 0:NC], AF.Sigmoid,
                                           bias=gb[(layer, "r")])
                      zc = gp.tile([GH, NC], F16, tag="z")
                      nc.scalar.activation(zc[:], prz[:, NC:], AF.Sigmoid,
                                           bias=gb[(layer, "zn")], scale=-1.0)
                      t_ = gp.tile([GH, NC], F16, tag="t_")
                      nc.vector.tensor_scalar(t_[:], r[:], gb[(layer, "hn")],
                                              None, OP.mult)
                      return dict(layer=layer, t=t, c=c, t0=True, pi=pi, t_=t_,
                                  nin=pi, id_done=False, zc=zc,
                                  ho=hwrite[:, c0:c1])
                  ph = ps.tile([GH, NC], F32, tag="ph")
                  # bias rows start both accumulations before h' exists
                  with hp():
                      nc.tensor.matmul(prz[:, 0:NC], brz[0:1, bo : bo + GH],
                                       onesm[:], start=True, stop=False)
                  nc.tensor.matmul(prz[:, NC:], brz[0:1, bo + GH : bo + 2 * GH],
                                   onesm[:], start=True, stop=False)
                  if kq is not None:
                      p0 = 32 * kq
                      tp = (p0, 0)
                      wk = wih[p0 : p0 + CO, :]
                      with hp():
                          nc.tensor.matmul(prz[:, 0:NC], whh[:, 0:GH], h,
                                           start=False, stop=False)
                          nc.tensor.matmul(prz[:, 0:NC], wk[:, 0:GH], xap,
                                           start=False, stop=True, tile_position=tp)
                      nc.tensor.matmul(prz[:, NC:], whh[:, GH : 2 * GH], h,
                                       start=False, stop=False)
                      nc.tensor.matmul(prz[:, NC:], wk[:, GH : 2 * GH], xap,
                                       start=False, stop=True, tile_position=tp)
                      nc.tensor.matmul(ph[:], whh[:, 2 * GH : 3 * GH], h)
                      nc.tensor.matmul(pi[:], wk[:, 2 * GH : 3 * GH], xap,
                                       start=True, stop=False,
                                       tile_position=tp)
                  else:
                      nc.tensor.matmul(prz[:, 0:NC], whh[:, 0:GH], h,
                                       start=False, stop=False)
                      nc.tensor.matmul(prz[:, 0:NC], wih[:, 0:GH], xap,
                                       start=False, stop=True)
                      nc.tensor.matmul(prz[:, NC:], whh[:, GH : 2 * GH], h,
                                       start=False, stop=False)
                      nc.tensor.matmul(prz[:, NC:], wih[:, GH : 2 * GH], xap,
                                       start=False, stop=True)
                      nc.tensor.matmul(ph[:], whh[:, 2 * GH : 3 * GH], h)
                      nc.tensor.matmul(pi[:], wih[:, 2 * GH : 3 * GH], xap,
                                       start=True, stop=(t < MN - 2))
                  rz = gp.tile([GH, 2 * NC], F16, tag="rz")
                  # sigma/stt/add free the PSUM banks PE needs to run ahead:
                  # high priority for BOTH layers keeps the bank rotation hot
                  with tc.high_priority():
                      nc.scalar.activation(rz[:], prz[:], AF.Sigmoid)
                  r = rz[:, 0:NC]
                  z = rz[:, NC:]
                  t_ = gp.tile([GH, NC], F16, tag="t_")
                  with tc.high_priority():
                      nc.vector.scalar_tensor_tensor(
                          t_[:], ph[:], gb[(layer, "hn")], r, OP.add, OP.mult
                      )
                  if layer == 1 and t < MN - 2:
                      # l1 has slack: plain DVE add keeps PE lighter
                      nin = gp.tile([GH, NC], F16, tag="s_")
                      with tc.high_priority():
                          nc.vector.tensor_add(nin[:], pi[:], t_[:])
                  else:
                      nin = pi
                  zm = up.tile([GH, NC], F16, tag="zm")
                  nc.vector.tensor_scalar(zm[:], z, 1.0, None, OP.subtract)
                  u_ = up.tile([GH, NC], F16, tag="u_")
                  nc.gpsimd.tensor_mul(u_[:], z, h)
                  return dict(layer=layer, t=t, c=c, t0=False, pi=pi, t_=t_,
                              nin=nin, id_done=(layer == 1 and t < MN - 2),
                              zm=zm, u_=u_, ho=hwrite[:, c0:c1])

              # PE accumulates t_ into the pi bank (replaces a 658ns DVE
              # add with a 213ns identity matmul); must be emitted before
              # the NEXT cell's pi matmul to keep the in-order PE queue
              # deadlock-free
              idq = []

              def flush_ids():
                  while idq:
                      ctx = idq.pop(0)
                      if not ctx["id_done"]:
                          nc.tensor.matmul(ctx["pi"][:], ident[:],
                                           ctx["t_"][:],
                                           start=False, stop=True)
                          ctx["id_done"] = True

              def cell_back(ctx):
                  layer = ctx["layer"]
                  hp = tc.high_priority if layer == 0 else _noprio
                  if not ctx["id_done"]:
                      nc.tensor.matmul(ctx["pi"][:], ident[:], ctx["t_"][:],
                                       start=False, stop=True)
                      ctx["id_done"] = True
                  n_ = gp.tile([GH, NC], F16, tag="n_")
                  with hp():
                      nc.scalar.activation(n_[:], ctx["nin"][:], AF.Tanh,
                                           bias=gb[(layer, "in")])
                  if ctx["t0"]:
                      nc.vector.tensor_mul(ctx["ho"], ctx["zc"][:], n_[:])
                      return
                  v_ = up.tile([GH, NC], F16, tag="v_")
                  with hp():
                      nc.vector.tensor_mul(v_[:], ctx["zm"][:], n_[:])
                  if layer == 0:
                      with hp():
                          nc.vector.tensor_sub(ctx["ho"], ctx["u_"][:], v_[:])
                  elif ctx["t"] >= MN - 2:
                      nc.vector.tensor_sub(ctx["ho"], ctx["u_"][:], v_[:])
                  else:
                      nc.gpsimd.tensor_sub(ctx["ho"], ctx["u_"][:], v_[:])

              # ---- main software-pipelined loop ----------------------------
              pend = []
              part = fp_.tile([GH, 3 * BLOC], F32, tag="part")  # [128, 96]
              n1ps = {}

              def chunk_done(c):
                  """h2 chunk c is final: emit its partial route-reduce and
                  the h2-dependent halves of the phase-C n1 matmuls."""
                  c0, c1 = c * NC, (c + 1) * NC
                  h2c = h2[:, c0:c1].rearrange("p (g s) -> p g s", s=16)
                  with tc.high_priority():
                      nc.vector.tensor_reduce(
                          part[:, 32 * c : 32 * (c + 1)],
                          h2c, mybir.AxisListType.X, OP.add,
                      )
                  p1 = ps.tile([128, 2 * NC], F32, tag="prz")
                  for m in range(2):
                      nc.tensor.matmul(
                          p1[:, m * NC : (m + 1) * NC],
                          W["wn1a"][:, 128 * m : 128 * (m + 1)], h2[:, c0:c1],
                          start=True, stop=False,
                      )
                  n1ps[c] = p1

              pend = []

              def _drain(q, depth):
                  while len(q) > depth:
                      done = q.pop(0)
                      cell_back(done)
                      if done["layer"] == 1 and done["t"] == MN - 1:
                          chunk_done(done["c"])

              def emit_cell(t, layer, c):
                  flush_ids()
                  ctx = cell_front(t, layer, c)
                  pend.append(ctx)
                  idq.append(ctx)
                  if len(pend) > PDEPTH:
                      _drain(pend, 2)

              # Emission plan: per step, l0(t) x3 + l1(t-1) x3, except one l1
              # cell is deferred from each phase-A-filler step (t % 4 == 2)
              # into the following filler-less step so its ACT work fills
              # that step's pipeline bubble.  A deferred l1(s) cell lands at
              # the START of step s+2, still before the l0(s+2) fronts that
              # overwrite the h1 buffer it reads.
              plan = {t: [] for t in range(MN + 1)}
              for t in range(MN):
                  for c in range(NCH):
                      plan[t].append((t, 0, c))
                      if t > 0:
                          plan[t].append((t - 1, 1, c))
              for t in (2, 6, 10, 14, 18, 19, 20, 21, 22):
                  # move the last l1 cell of step t's emission to the head
                  # of step t+1 (deferral by one step: h1 still valid)
                  cell = plan[t].pop()
                  assert cell[1] == 1
                  plan[t + 1].insert(0, cell)

              for sb in range(NCH):
                  emitA(0, sb)
              for t in range(MN):
                  gnext, sb = t // 4 + 1, t % 4
                  if gnext < NG:
                      if sb == 0:
                          for n in range(4 * gnext, 4 * gnext + 4):
                              load_xc(n)
                      if sb >= 1:
                          emitA(gnext, sb - 1)
                  for cell in plan[t]:
                      emit_cell(*cell)
                  if t == 0:
                      # layer-1 + later layer-0 weights stream in behind the
                      # first step's compute
                      W["whh0"] = wtile(d_whh0, [GH, 3 * GH], F16)
                      W["wih1"] = wtile(d_wih1, [GH, 3 * GH], F16)
                      W["whh1"] = wtile(d_whh1, [GH, 3 * GH], F16)
                  elif t == 2:
                      W["wn1a"] = wtile(d_wn1a, [GH, 256], F16)
                      W["wn1b"] = wtile(d_wn1b, [GH, 256], F16)
                      W["bn1r"] = wtile(d_bn1r, [1, 256], F16)
                      W["wn2a"] = wtile(d_wn2a, [128, 128], F16)
                      W["wn2b"] = wtile(d_wn2b, [128, 128], F16)
                      W["bn2"] = wtile(d_bn2, [128, 1], F32)
                      W["wn3"] = wtile(d_wn3, [GH, MN], F16)
                      W["bn3"] = wtile(d_bn3, [1, MN], F32)
                      W["sel"] = wtile(d_sel, [BLOC, S], F16)
                      W["mrn"] = wtile(d_mrn, [128, MN + NQ], F32)
                      W["ones128"] = wtile(d_ones, [1, 128], F32)
                  elif t == 3:
                      # precompute all 12 softmax masks off the tail
                      for j in range(NQ):
                          nc.vector.tensor_scalar(
                              mskall[:, j * MN : (j + 1) * MN],
                              W["mrn"][:, 0:MN],
                              W["mrn"][:, MN + j : MN + j + 1],
                              None, OP.is_lt,
                          )
              for c in range(NCH):
                  emit_cell(MN - 1, 1, c)
              _drain(pend, 0)

              # ---- phase C: route mean + node MLP + masked softmax ---------
              # dummy exp: loads the exp table set (covers tanh+relu too)
              # during the mean-chain window instead of on the exp path
              nc.scalar.activation(warm2[:], warm[:], AF.Exp)
              mean32 = fp_.tile([GH, BLOC], F32, tag="mean32")
              partv = part[:].rearrange("p (b j) -> p b j", j=3)
              with tc.high_priority():
                  nc.vector.tensor_reduce(mean32[:], partv, mybir.AxisListType.X, OP.add)
              mean = fp_.tile([GH, BLOC], F16, tag="mean")
              with tc.high_priority():
                  nc.vector.tensor_copy(mean[:], mean32[:])
              pmt = ps.tile([BLOC, 256], F32, tag="pi")
              # keep PE's p-state warm through the mean chain: no-dep dummy
              # matmuls into pmt's bank, reset by the real start=True matmul
              for _ in range(10):
                  nc.tensor.matmul(pmt[0:1, :], onesm[0:1, 0:1], onesm[0:1, 0:256])
              with tc.high_priority():
                  nc.tensor.matmul(pmt[:], mean[:], W["wn1b"][:])
              mmt = fp_.tile([BLOC, 256], F16, tag="mmt")
              with tc.high_priority():
                  nc.vector.tensor_copy(mmt[:], pmt[:])

              for c in range(NCH):
                  c0, c1 = c * NC, (c + 1) * NC
                  p1 = n1ps[c]
                  for m in range(2):
                      half = p1[:, m * NC : (m + 1) * NC]
                      nc.tensor.matmul(
                          half, W["bn1r"][0:1, 128 * m : 128 * (m + 1)],
                          onesm[:], start=False, stop=False,
                      )
                      nc.tensor.matmul(
                          half, mmt[:, 128 * m : 128 * (m + 1)], W["sel"][:, c0:c1],
                          start=False, stop=True,
                      )
                  a1 = fp_.tile([128, 2 * NC], F16, tag="n1")
                  nc.scalar.activation(a1[:], p1[:], AF.Relu)
                  p2 = ps.tile([128, NC], F32, tag="ph")
                  nc.tensor.matmul(p2[:], W["wn2a"][:], a1[:, 0:NC],
                                   start=True, stop=False)
                  nc.tensor.matmul(p2[:], W["wn2b"][:], a1[:, NC:],
                                   start=False, stop=True)
                  n2 = fp_.tile([128, NC], F16, tag="n2")
                  nc.scalar.activation(n2[:], p2[:], AF.Relu, bias=W["bn2"][:])
                  # all 4 q-groups' logits side by side in one PSUM bank:
                  # one 96-wide exp + one grouped denominator reduce
                  pl = ps.tile([128, 4 * MN], F32, tag="pi" if c % 2 == 0 else "ph")
                  for q in range(NC // 128):
                      sl = pl[:, q * MN : (q + 1) * MN]
                      nc.tensor.matmul(sl, W["ones128"][:], W["bn3"][0:1, 0:MN],
                                       start=True, stop=False)
                      nc.tensor.matmul(
                          sl, n2[:, q * 128 : (q + 1) * 128], W["wn3"][:],
                          start=False, stop=True,
                      )
                  ex = fp_.tile([128, 4 * MN], F32, tag="ex")
                  nc.scalar.activation(ex[:], pl[:], AF.Exp)
                  exv = ex[:].rearrange("p (q m) -> p q m", m=MN)
                  sm = fp_.tile([128, 4], F32, tag="sm")
                  nc.vector.tensor_reduce(sm[:], exv, mybir.AxisListType.X, OP.add)
                  rec = fp_.tile([128, 4], F32, tag="rec")
                  nc.vector.reciprocal(rec[:], sm[:])
                  for q in range(NC // 128):
                      j = c * 4 + q
                      nc.vector.scalar_tensor_tensor(
                          stage[:, j * MN : (j + 1) * MN],
                          ex[:, q * MN : (q + 1) * MN], rec[:, q : q + 1],
                          mskall[:, j * MN : (j + 1) * MN], OP.mult, OP.mult,
                      )
                  nc.sync.dma_start(
                      d_out.ap()[:, c * 4 * MN : (c + 1) * 4 * MN],
                      stage[:, c * 4 * MN : (c + 1) * 4 * MN],
                  )

    nc.compile()
    return nc


def _prep_inputs(inputs):
    """Host-side preprocessing -> list of per-core input dicts."""
    state = np.ascontiguousarray(inputs["state"], dtype=np.float32)
    rn = state[:, :MR]                                    # [B, 48]
    cust = state[:, MR:].reshape(B, MR, MN, FEAT)

    def f32(x):
        return np.ascontiguousarray(np.asarray(x, dtype=np.float32))

    Wih0 = f32(inputs["Wih0"]); Whh0 = f32(inputs["Whh0"])
    Wih1 = f32(inputs["Wih1"]); Whh1 = f32(inputs["Whh1"])
    bih0 = f32(inputs["bih0"]); bhh0 = f32(inputs["bhh0"])
    bih1 = f32(inputs["bih1"]); bhh1 = f32(inputs["bhh1"])
    bn1 = f32(inputs["bn1"])

    bcols = {}
    for layer, bih, bhh in ((0, bih0, bhh0), (1, bih1, bhh1)):
        bcols[(layer, "r")] = bih[0:GH] + bhh[0:GH]
        bcols[(layer, "z")] = np.zeros(GH, np.float32)       # pre-added via PE
        bcols[(layer, "zn")] = -(bih[GH : 2 * GH] + bhh[GH : 2 * GH])
        bcols[(layer, "in")] = bih[2 * GH :]
        bcols[(layer, "hn")] = bhh[2 * GH :]
    # The r-gate bias is pre-added via PE for t>0 (brz) AND used as the ACT
    # bias at t=0; to avoid double-adding, brz carries it for t>0 merged
    # sigmoids (which pass no ACT bias), while gball's "r" column is used
    # only by the t=0 path.  Same bias value in both places is correct.
    gball = np.stack([bcols[key] for key in _GBCOLS], axis=1)
    brz = np.concatenate([
        bih0[GH : 2 * GH] * 0 + (bih0[0:GH] + bhh0[0:GH]),   # b0_r
        (bih0[GH : 2 * GH] + bhh0[GH : 2 * GH]),             # b0_z
        (bih1[0:GH] + bhh1[0:GH]),                           # b1_r
        (bih1[GH : 2 * GH] + bhh1[GH : 2 * GH]),             # b1_z
    ]).reshape(1, 4 * GH)

    com = {
        "Wc1h": np.ascontiguousarray(np.asarray(inputs["Wc1"], np.float16)),
        "bc1": f32(inputs["bc1"]).reshape(CH, 1),
        "Wc2h": np.ascontiguousarray(np.asarray(inputs["Wc2"], np.float16)),
        "bc2s": np.tile(f32(inputs["bc2"]).reshape(CO), 4).reshape(128, 1),
        "Wih0h": np.ascontiguousarray(np.tile(np.asarray(Wih0, np.float16), (4, 1))),
        "Whh0h": Whh0.astype(np.float16), "Wih1h": Wih1.astype(np.float16),
        "Whh1h": Whh1.astype(np.float16),
        "gball": np.ascontiguousarray(gball),
        "brz": brz.astype(np.float16),
        "onesm": np.ones((1, NC), np.float16),
        "Wn1a": f32(inputs["Wn1"])[0:GH, :].astype(np.float16),
        "Wn1b": (f32(inputs["Wn1"])[GH:, :] / np.float32(MR)).astype(np.float16),
        "bn1row": bn1.reshape(1, 256).astype(np.float16),
        "Wn2a": f32(inputs["Wn2"])[0:128, :].astype(np.float16),
        "Wn2b": f32(inputs["Wn2"])[128:256, :].astype(np.float16),
        "bn2c": f32(inputs["bn2"]).reshape(128, 1),
        "Wn3h": np.asarray(inputs["Wn3"], np.float16),
        "bn3r": f32(inputs["bn3"]).reshape(1, MN),
        "ones128": np.ones((1, 128), np.float32),
        "ident128": np.eye(128, dtype=np.float16),
    }
    sel = np.zeros((BLOC, S), np.float32)
    sel[np.arange(S) // MR, np.arange(S)] = 1.0
    com["sel"] = sel.astype(np.float16)

    iota = np.tile(np.arange(MN, dtype=np.float32), (128, 1))

    in_maps = []
    for core in range(NCORES):
        b0, b1 = core * BLOC, (core + 1) * BLOC
        # cust_fm[f, n*S + (b*MR+r)] = cust[b, r, n, f]
        cfm = cust[b0:b1].transpose(3, 2, 0, 1).reshape(FEAT, MN * S)
        m = dict(com)
        m["cust_fm"] = np.ascontiguousarray(cfm.astype(np.float16))
        rnc = rn[b0:b1].reshape(S)[: NQ * 128].reshape(NQ, 128).T
        m["mrn"] = np.ascontiguousarray(np.concatenate([iota, rnc], axis=1))
        in_maps.append(m)
    return in_maps


def _run(inputs, **kw):
    if "nc" not in _cache:
        _cache["nc"] = _build()
    nc = _cache["nc"]
    in_maps = _prep_inputs(inputs)
    return run_bass_kernel_spmd(nc, in_maps, core_ids=list(range(NCORES)), **kw)


def kernel(**inputs) -> np.ndarray:
    res = _run(inputs)
    outs = []
    for r in res.results:
        o = r["out_tm"]                        # [128, NQ*MN]
        o = o.reshape(128, NQ, MN).transpose(1, 0, 2).reshape(S, MN)
        outs.append(o)
    return np.concatenate(outs, axis=0).reshape(B, MR, MN)


# revision 66
# speedup vs baseline: 1.0106x; 1.0106x over previous
"""Trainium2 Bass kernel for nn_MLP_Route_RL_Model (route RL model).

Reference math (per batch element b of 256):
  - state = [route_nums (48) | customers (48*24*36)]
  - customer MLP (tanh-tanh, 36->128->32) on every node of every route
  - 2-layer GRU (hidden 128) over the 24 nodes of each of the 48 routes
  - route summary mean, node-selection MLP 256->256->128->24, masked softmax

Sharding: pure data parallel over batch B=256 -> 8 cores x 32.

Schedule: the two GRU layers are software-pipelined with layer 1 lagging
layer 0 by one step (h1 double-buffered), so the 6 cells emitted per step
(3 chunks x 2 layers) are mutually independent.  Each cell is split into a
"front" (matmuls + merged r|z sigmoid + n-gate input prep) and a "back"
(tanh + hidden update), with back(k-2) emitted after front(k) so the
Activation engine never waits on the DVE chain of the cell it just gated.
The r and z gates share one 2-bank PSUM tile and one 1024-wide sigmoid;
their biases are pre-added by K=1 PE matmuls (bias row x ones).
Customer-MLP groups are interleaved one sub-block per step as filler, with
node pairs sharing one 1024-wide tanh.
"""

import contextlib
import os as _os
PDEPTH = int(_os.environ.get("PDEPTH", "2"))
import sys

import numpy as np

sys.path.insert(0, "/opt/trn_rl_repo")

import concourse.bass as bass  # noqa: E402
import concourse.bacc as bacc  # noqa: E402
import concourse.mybir as mybir  # noqa: E402
import concourse.tile as tile  # noqa: E402
from concourse.bass_utils import run_bass_kernel_spmd  # noqa: E402

F32 = mybir.dt.float32
F16 = mybir.dt.float16
AF = mybir.ActivationFunctionType
OP = mybir.AluOpType

# Problem shape constants
B = 256
NCORES = 8
BLOC = B // NCORES          # 32 batch rows per core
MR = 48                     # routes per batch
MN = 24                     # nodes per route
FEAT = 36
CH = 128                    # customer hidden
CO = 32                     # customer out
GH = 128                    # GRU hidden
S = BLOC * MR               # sequences per core = 1536
NC = 512                    # token chunk (PSUM bank width in fp32)
NCH = S // NC               # chunks per core = 3
NG = MN // 4                # node groups of 4 (cust_out partition stacking)
NQ = S // 128               # 128-token output groups = 12

_GBCOLS = [(l, g) for l in (0, 1) for g in ("r", "z", "zn", "in", "hn")]

_cache = {}


@contextlib.contextmanager
def _noprio():
    yield


def _build(reps=1):
    """Trace + schedule the per-core Tile kernel. Returns the Bass module."""
    nc = bacc.Bacc("TRN2", target_bir_lowering=False, debug=False)

    # ---- DRAM I/O ----------------------------------------------------------
    d_cust = nc.dram_tensor("cust_fm", [FEAT, MN * S], F16, kind="ExternalInput")
    d_wc1 = nc.dram_tensor("Wc1h", [FEAT, CH], F16, kind="ExternalInput")
    d_bc1 = nc.dram_tensor("bc1", [CH, 1], F32, kind="ExternalInput")
    d_wc2 = nc.dram_tensor("Wc2h", [CH, CO], F16, kind="ExternalInput")
    d_bc2 = nc.dram_tensor("bc2s", [128, 1], F32, kind="ExternalInput")
    d_wih0 = nc.dram_tensor("Wih0h", [128, 3 * GH], F16, kind="ExternalInput")
    d_whh0 = nc.dram_tensor("Whh0h", [GH, 3 * GH], F16, kind="ExternalInput")
    d_wih1 = nc.dram_tensor("Wih1h", [GH, 3 * GH], F16, kind="ExternalInput")
    d_whh1 = nc.dram_tensor("Whh1h", [GH, 3 * GH], F16, kind="ExternalInput")
    # all 10 GRU gate bias columns in one tensor (single DMA)
    d_gball = nc.dram_tensor("gball", [GH, 10], F32, kind="ExternalInput")
    # r|z bias rows for the PE pre-add: [b0_r|b0_z|b1_r|b1_z]
    d_brz = nc.dram_tensor("brz", [1, 4 * GH], F16, kind="ExternalInput")
    d_onesm = nc.dram_tensor("onesm", [1, NC], F16, kind="ExternalInput")
    d_wn1a = nc.dram_tensor("Wn1a", [GH, 256], F16, kind="ExternalInput")
    d_wn1b = nc.dram_tensor("Wn1b", [GH, 256], F16, kind="ExternalInput")
    d_bn1r = nc.dram_tensor("bn1row", [1, 256], F16, kind="ExternalInput")
    d_wn2a = nc.dram_tensor("Wn2a", [128, 128], F16, kind="ExternalInput")
    d_wn2b = nc.dram_tensor("Wn2b", [128, 128], F16, kind="ExternalInput")
    d_bn2 = nc.dram_tensor("bn2c", [128, 1], F32, kind="ExternalInput")
    d_wn3 = nc.dram_tensor("Wn3h", [GH, MN], F16, kind="ExternalInput")
    d_bn3 = nc.dram_tensor("bn3r", [1, MN], F32, kind="ExternalInput")
    d_sel = nc.dram_tensor("sel", [BLOC, S], F16, kind="ExternalInput")
    # iota (24 cols) | per-q-group route_nums (12 cols)
    d_mrn = nc.dram_tensor("mrn", [128, MN + NQ], F32, kind="ExternalInput")
    d_ident = nc.dram_tensor("ident128", [128, 128], F16, kind="ExternalInput")
    d_ones = nc.dram_tensor("ones128", [1, 128], F32, kind="ExternalInput")
    d_out = nc.dram_tensor("out_tm", [128, NQ * MN], F32, kind="ExternalOutput")

    with tile.TileContext(nc) as tc:
        with (
            tc.tile_pool(name="wpool", bufs=1) as wp,
            tc.tile_pool(name="state", bufs=1) as sp,
            tc.tile_pool(name="xin", bufs=10) as xp,
            tc.tile_pool(name="h1c", bufs=4) as h1p,
            tc.tile_pool(name="gates", bufs=8) as gp,
            tc.tile_pool(name="upd", bufs=8) as up,
            tc.tile_pool(name="fin", bufs=4) as fp_,
            tc.tile_pool(name="ps", bufs=2, space="PSUM") as ps,
        ):
            def wtile(dram, shape, dtype):
                t = wp.tile(shape, dtype, tag=dram.name)
                nc.sync.dma_start(t[:], dram.ap())
                return t

            # ---- phase-A-critical weights first (unblocks first tanh) ------
            wc1 = wtile(d_wc1, [FEAT, CH], F16)
            bc1 = wtile(d_bc1, [CH, 1], F32)
            wc2 = wtile(d_wc2, [CH, CO], F16)
            bc2 = wtile(d_bc2, [128, 1], F32)

            xtiles = {}

            def load_xc(n, eng=None):
                if n not in xtiles:
                    xn = xp.tile([FEAT, S], F16, tag="xc", name=f"xc{n}")
                    (eng or nc.sync).dma_start(
                        xn[:], d_cust.ap()[:, n * S : (n + 1) * S]
                    )
                    xtiles[n] = xn
                return xtiles[n]

            # first group's customer tiles go over SWDGE (Pool queue) in
            # parallel with the weight DMAs on the HWDGE path
            for n in range(4):
                load_xc(n, eng=nc.gpsimd)

            # warm the ACT function tables during the DMA-wait dead time at
            # the head.  tanh+sigmoid+relu share one table set, so this
            # costs a single load; exp is NOT warmed here (its set lacks
            # sigmoid and would force a reload before the GRU) — a dummy
            # exp after the GRU drain loads the exp set (which also covers
            # tanh+relu) during the idle mean-reduce window.
            warm = wp.tile([1, 1], F32, tag="warm")
            warm2 = wp.tile([1, 1], F32, tag="warm2")
            nc.vector.memset(warm[:], 0.0)
            for fn in (AF.Tanh, AF.Sigmoid, AF.Relu):
                nc.scalar.activation(warm2[:], warm[:], fn)

            # persistent state
            cust = sp.tile([128, NG * S], F16, tag="cust_out")
            h1a = sp.tile([GH, S], F16, tag="h1a")
            h1b = sp.tile([GH, S], F16, tag="h1b")
            h2 = sp.tile([GH, S], F16, tag="h2")
            stage = sp.tile([128, NQ * MN], F32, tag="stage")
            mskall = sp.tile([128, NQ * MN], F32, tag="mskall")

            # ---- layer-0 GRU weights (needed by step 0) --------------------
            wih0 = wtile(d_wih0, [128, 3 * GH], F16)
            gball = wtile(d_gball, [GH, 10], F32)
            brz = wtile(d_brz, [1, 4 * GH], F16)
            onesm = wtile(d_onesm, [1, NC], F16)
            ident = wtile(d_ident, [128, 128], F16)
            gb = {key: gball[:, i : i + 1] for i, key in enumerate(_GBCOLS)}

            W = {}  # late-loaded weights (filled during the step loop)

          # timing-calibration repeat loop (reps=1 in production)
          # fmt: off
            for _rep in range(reps):
              # ---- phase A: customer MLP (one (group, sub-block) slice) ----
              def emitA(g, sb):
                  c2 = ps.tile([128, NC], F32, tag="pi")
                  for kp in (0, 1):
                      pp = ps.tile([CH, 2 * NC], F32, tag="prz")
                      for i in (0, 1):
                          n = 4 * g + 2 * kp + i
                          nc.tensor.matmul(
                              pp[:, i * NC : (i + 1) * NC], wc1[:],
                              xtiles[n][:, sb * NC : (sb + 1) * NC],
                          )
                      hp2 = h1p.tile([CH, 2 * NC], F16, tag="h1c")
                      nc.scalar.activation(hp2[:], pp[:], AF.Tanh, bias=bc1[:])
                      for i in (0, 1):
                          k = 2 * kp + i
                          nc.tensor.matmul(
                              c2[32 * k : 32 * (k + 1), :], wc2[:],
                              hp2[:, i * NC : (i + 1) * NC],
                              tile_position=(0, 32 * k),
                          )
                  nc.scalar.activation(
                      cust[:, g * S + sb * NC : g * S + (sb + 1) * NC],
                      c2[:], AF.Tanh, bias=bc2[:],
                  )

              # ---- GRU cell, split into front (gates) and back (update) ----
              def cell_front(t, layer, c):
                  c0, c1 = c * NC, (c + 1) * NC
                  if layer == 0:
                      g, k = t // 4, t % 4
                      xap = cust[32 * k : 32 * (k + 1), g * S + c0 : g * S + c1]
                      kq = k
                      wih, whh = wih0, W.get("whh0")
                      hread = h1b if t % 2 == 0 else h1a
                      hwrite = h1a if t % 2 == 0 else h1b
                  else:
                      xap = (h1a if t % 2 == 0 else h1b)[:, c0:c1]
                      kq = None
                      wih, whh = W.get("wih1"), W.get("whh1")
                      hread = hwrite = h2
                  h = hread[:, c0:c1]
                  hp = tc.high_priority if layer == 0 else _noprio
                  bo = 2 * GH * layer
                  prz = ps.tile([GH, 2 * NC], F32, tag="prz")
                  pi = ps.tile([GH, NC], F32, tag="pi")
                  if t == 0:
                      # h == 0: whh terms vanish; separate sigmoids with the
                      # ACT bias operand (1-z via sigmoid(-x)).
                      if kq is not None:
                          p0 = 32 * kq
                          tp = (p0, 0)
                          wk = wih[p0 : p0 + CO, :]
                          nc.tensor.matmul(prz[:, 0:NC], wk[:, 0:GH], xap,
                                           tile_position=tp)
                          nc.tensor.matmul(prz[:, NC:], wk[:, GH : 2 * GH], xap,
                                           tile_position=tp)
                          nc.tensor.matmul(pi[:], wk[:, 2 * GH : 3 * GH], xap,
                                           start=True, stop=False,
                                           tile_position=tp)
                      else:
                          nc.tensor.matmul(prz[:, 0:NC], wih[:, 0:GH], xap)
                          nc.tensor.matmul(prz[:, NC:], wih[:, GH : 2 * GH], xap)
                          nc.tensor.matmul(pi[:], wih[:, 2 * GH : 3 * GH], xap,
                                           start=True, stop=False)
                      r = gp.tile([GH, NC], F16, tag="r")
                      nc.scalar.activation(r[:], prz[:, 0:NC], AF.Sigmoid,
                                           bias=gb[(layer, "r")])
                      zc = gp.tile([GH, NC], F16, tag="z")
                      nc.scalar.activation(zc[:], prz[:, NC:], AF.Sigmoid,
                                           bias=gb[(layer, "zn")], scale=-1.0)
                      t_ = gp.tile([GH, NC], F16, tag="t_")
                      nc.vector.tensor_scalar(t_[:], r[:], gb[(layer, "hn")],
                                              None, OP.mult)
                      return dict(layer=layer, t=t, c=c, t0=True, pi=pi, t_=t_,
                                  nin=pi, id_done=False, zc=zc,
                                  ho=hwrite[:, c0:c1])
                  ph = ps.tile([GH, NC], F32, tag="ph")
                  # bias rows start both accumulations before h' exists
                  with hp():
                      nc.tensor.matmul(prz[:, 0:NC], brz[0:1, bo : bo + GH],
                                       onesm[:], start=True, stop=False)
                  nc.tensor.matmul(prz[:, NC:], brz[0:1, bo + GH : bo + 2 * GH],
                                   onesm[:], start=True, stop=False)
                  if kq is not None:
                      p0 = 32 * kq
                      tp = (p0, 0)
                      wk = wih[p0 : p0 + CO, :]
                      with hp():
                          nc.tensor.matmul(prz[:, 0:NC], whh[:, 0:GH], h,
                                           start=False, stop=False)
                          nc.tensor.matmul(prz[:, 0:NC], wk[:, 0:GH], xap,
                                           start=False, stop=True, tile_position=tp)
                      nc.tensor.matmul(prz[:, NC:], whh[:, GH : 2 * GH], h,
                                       start=False, stop=False)
                      nc.tensor.matmul(prz[:, NC:], wk[:, GH : 2 * GH], xap,
                                       start=False, stop=True, tile_position=tp)
                      nc.tensor.matmul(ph[:], whh[:, 2 * GH : 3 * GH], h)
                      nc.tensor.matmul(pi[:], wk[:, 2 * GH : 3 * GH], xap,
                                       start=True, stop=False,
                                       tile_position=tp)
                  else:
                      nc.tensor.matmul(prz[:, 0:NC], whh[:, 0:GH], h,
                                       start=False, stop=False)
                      nc.tensor.matmul(prz[:, 0:NC], wih[:, 0:GH], xap,
                                       start=False, stop=True)
                      nc.tensor.matmul(prz[:, NC:], whh[:, GH : 2 * GH], h,
                                       start=False, stop=False)
                      nc.tensor.matmul(prz[:, NC:], wih[:, GH : 2 * GH], xap,
                                       start=False, stop=True)
                      nc.tensor.matmul(ph[:], whh[:, 2 * GH : 3 * GH], h)
                      nc.tensor.matmul(pi[:], wih[:, 2 * GH : 3 * GH], xap,
                                       start=True, stop=(t < MN - 2))
                  rz = gp.tile([GH, 2 * NC], F16, tag="rz")
                  # sigma/stt/add free the PSUM banks PE needs to run ahead:
                  # high priority for BOTH layers keeps the bank rotation hot
                  with tc.high_priority():
                      nc.scalar.activation(rz[:], prz[:], AF.Sigmoid)
                  r = rz[:, 0:NC]
                  z = rz[:, NC:]
                  t_ = gp.tile([GH, NC], F16, tag="t_")
                  with tc.high_priority():
                      nc.vector.scalar_tensor_tensor(
                          t_[:], ph[:], gb[(layer, "hn")], r, OP.add, OP.mult
                      )
                  if layer == 1 and t < MN - 2:
                      # l1 has slack: plain DVE add keeps PE lighter
                      nin = gp.tile([GH, NC], F16, tag="s_")
                      with tc.high_priority():
                          nc.vector.tensor_add(nin[:], pi[:], t_[:])
                  else:
                      nin = pi
                  zm = up.tile([GH, NC], F16, tag="zm")
                  nc.vector.tensor_scalar(zm[:], z, 1.0, None, OP.subtract)
                  u_ = up.tile([GH, NC], F16, tag="u_")
                  nc.gpsimd.tensor_mul(u_[:], z, h)
                  return dict(layer=layer, t=t, c=c, t0=False, pi=pi, t_=t_,
                              nin=nin, id_done=(layer == 1 and t < MN - 2),
                              zm=zm, u_=u_, ho=hwrite[:, c0:c1])

              # PE accumulates t_ into the pi bank (replaces a 658ns DVE
              # add with a 213ns identity matmul); must be emitted before
              # the NEXT cell's pi matmul to keep the in-order PE queue
              # deadlock-free
              idq = []

              def flush_ids():
                  while idq:
                      ctx = idq.pop(0)
                      if not ctx["id_done"]:
                          nc.tensor.matmul(ctx["pi"][:], ident[:],
                                           ctx["t_"][:],
                                           start=False, stop=True)
                          ctx["id_done"] = True

              def cell_back(ctx):
                  layer = ctx["layer"]
                  hp = tc.high_priority if layer == 0 else _noprio
                  if not ctx["id_done"]:
                      nc.tensor.matmul(ctx["pi"][:], ident[:], ctx["t_"][:],
                                       start=False, stop=True)
                      ctx["id_done"] = True
                  n_ = gp.tile([GH, NC], F16, tag="n_")
                  with hp():
                      nc.scalar.activation(n_[:], ctx["nin"][:], AF.Tanh,
                                           bias=gb[(layer, "in")])
                  if ctx["t0"]:
                      nc.vector.tensor_mul(ctx["ho"], ctx["zc"][:], n_[:])
                      return
                  v_ = up.tile([GH, NC], F16, tag="v_")
                  with hp():
                      nc.vector.tensor_mul(v_[:], ctx["zm"][:], n_[:])
                  if layer == 0:
                      with hp():
                          nc.vector.tensor_sub(ctx["ho"], ctx["u_"][:], v_[:])
                  elif ctx["t"] >= MN - 2:
                      nc.vector.tensor_sub(ctx["ho"], ctx["u_"][:], v_[:])
                  else:
                      nc.gpsimd.tensor_sub(ctx["ho"], ctx["u_"][:], v_[:])

              # ---- main software-pipelined loop ----------------------------
              pend = []
              part = fp_.tile([GH, 3 * BLOC], F32, tag="part")  # [128, 96]
              n1ps = {}

              def chunk_done(c):
                  """h2 chunk c is final: emit its partial route-reduce and
                  the h2-dependent halves of the phase-C n1 matmuls."""
                  c0, c1 = c * NC, (c + 1) * NC
                  h2c = h2[:, c0:c1].rearrange("p (g s) -> p g s", s=16)
                  with tc.high_priority():
                      nc.vector.tensor_reduce(
                          part[:, 32 * c : 32 * (c + 1)],
                          h2c, mybir.AxisListType.X, OP.add,
                      )
                  p1 = ps.tile([128, 2 * NC], F32, tag="prz")
                  for m in range(2):
                      nc.tensor.matmul(
                          p1[:, m * NC : (m + 1) * NC],
                          W["wn1a"][:, 128 * m : 128 * (m + 1)], h2[:, c0:c1],
                          start=True, stop=False,
                      )
                  n1ps[c] = p1

              pend = []

              def _drain(q, depth):
                  while len(q) > depth:
                      done = q.pop(0)
                      cell_back(done)
                      if done["layer"] == 1 and done["t"] == MN - 1:
                          chunk_done(done["c"])

              def emit_cell(t, layer, c):
                  flush_ids()
                  ctx = cell_front(t, layer, c)
                  pend.append(ctx)
                  idq.append(ctx)
                  if len(pend) > PDEPTH:
                      _drain(pend, 2)

              # Emission plan: per step, l0(t) x3 + l1(t-1) x3, except one l1
              # cell is deferred from each phase-A-filler step (t % 4 == 2)
              # into the following filler-less step so its ACT work fills
              # that step's pipeline bubble.  A deferred l1(s) cell lands at
              # the START of step s+2, still before the l0(s+2) fronts that
              # overwrite the h1 buffer it reads.
              plan = {t: [] for t in range(MN + 1)}
              for t in range(MN):
                  for c in range(NCH):
                      plan[t].append((t, 0, c))
                      if t > 0:
                          plan[t].append((t - 1, 1, c))
              for t in (2, 6, 10, 14, 18, 19, 20, 21, 22):
                  # move the last l1 cell of step t's emission to the head
                  # of step t+1 (deferral by one step: h1 still valid)
                  cell = plan[t].pop()
                  assert cell[1] == 1
                  plan[t + 1].insert(0, cell)

              for sb in range(NCH):
                  emitA(0, sb)
              for t in range(MN):
                  gnext, sb = t // 4 + 1, t % 4
                  if gnext < NG:
                      if sb == 0:
                          for n in range(4 * gnext, 4 * gnext + 4):
                              load_xc(n)
                      if sb < NCH:
                          emitA(gnext, sb)
                  for cell in plan[t]:
                      emit_cell(*cell)
                  if t == 0:
                      # layer-1 + later layer-0 weights stream in behind the
                      # first step's compute
                      W["whh0"] = wtile(d_whh0, [GH, 3 * GH], F16)
                      W["wih1"] = wtile(d_wih1, [GH, 3 * GH], F16)
                      W["whh1"] = wtile(d_whh1, [GH, 3 * GH], F16)
                  elif t == 2:
                      W["wn1a"] = wtile(d_wn1a, [GH, 256], F16)
                      W["wn1b"] = wtile(d_wn1b, [GH, 256], F16)
                      W["bn1r"] = wtile(d_bn1r, [1, 256], F16)
                      W["wn2a"] = wtile(d_wn2a, [128, 128], F16)
                      W["wn2b"] = wtile(d_wn2b, [128, 128], F16)
                      W["bn2"] = wtile(d_bn2, [128, 1], F32)
                      W["wn3"] = wtile(d_wn3, [GH, MN], F16)
                      W["bn3"] = wtile(d_bn3, [1, MN], F32)
                      W["sel"] = wtile(d_sel, [BLOC, S], F16)
                      W["mrn"] = wtile(d_mrn, [128, MN + NQ], F32)
                      W["ones128"] = wtile(d_ones, [1, 128], F32)
                  elif t == 3:
                      # precompute all 12 softmax masks off the tail
                      for j in range(NQ):
                          nc.vector.tensor_scalar(
                              mskall[:, j * MN : (j + 1) * MN],
                              W["mrn"][:, 0:MN],
                              W["mrn"][:, MN + j : MN + j + 1],
                              None, OP.is_lt,
                          )
              for c in range(NCH):
                  emit_cell(MN - 1, 1, c)
              _drain(pend, 0)

              # ---- phase C: route mean + node MLP + masked softmax ---------
              # dummy exp: loads the exp table set (covers tanh+relu too)
              # during the mean-chain window instead of on the exp path
              nc.scalar.activation(warm2[:], warm[:], AF.Exp)
              mean32 = fp_.tile([GH, BLOC], F32, tag="mean32")
              partv = part[:].rearrange("p (b j) -> p b j", j=3)
              with tc.high_priority():
                  nc.vector.tensor_reduce(mean32[:], partv, mybir.AxisListType.X, OP.add)
              mean = fp_.tile([GH, BLOC], F16, tag="mean")
              with tc.high_priority():
                  nc.vector.tensor_copy(mean[:], mean32[:])
              pmt = ps.tile([BLOC, 256], F32, tag="pi")
              # keep PE's p-state warm through the mean chain: no-dep dummy
              # matmuls into pmt's bank, reset by the real start=True matmul
              for _ in range(10):
                  nc.tensor.matmul(pmt[0:1, :], onesm[0:1, 0:1], onesm[0:1, 0:256])
              with tc.high_priority():
                  nc.tensor.matmul(pmt[:], mean[:], W["wn1b"][:])
              mmt = fp_.tile([BLOC, 256], F16, tag="mmt")
              with tc.high_priority():
                  nc.vector.tensor_copy(mmt[:], pmt[:])

              for c in range(NCH):
                  c0, c1 = c * NC, (c + 1) * NC
                  p1 = n1ps[c]
                  for m in range(2):
                      half = p1[:, m * NC : (m + 1) * NC]
                      nc.tensor.matmul(
                          half, W["bn1r"][0:1, 128 * m : 128 * (m + 1)],
                          onesm[:], start=False, stop=False,
                      )
                      nc.tensor.matmul(
                          half, mmt[:, 128 * m : 128 * (m + 1)], W["sel"][:, c0:c1],
                          start=False, stop=True,
                      )
                  a1 = fp_.tile([128, 2 * NC], F16, tag="n1")
                  nc.scalar.activation(a1[:], p1[:], AF.Relu)
                  p2 = ps.tile([128, NC], F32, tag="ph")
                  nc.tensor.matmul(p2[:], W["wn2a"][:], a1[:, 0:NC],
                                   start=True, stop=False)
                  nc.tensor.matmul(p2[:], W["wn2b"][:], a1[:, NC:],
                                   start=False, stop=True)
                  n2 = fp_.tile([128, NC], F16, tag="n2")
                  nc.scalar.activation(n2[:], p2[:], AF.Relu, bias=W["bn2"][:])
                  # all 4 q-groups' logits side by side in one PSUM bank:
                  # one 96-wide exp + one grouped denominator reduce
                  pl = ps.tile([128, 4 * MN], F32, tag="pi" if c % 2 == 0 else "ph")
                  for q in range(NC // 128):
                      sl = pl[:, q * MN : (q + 1) * MN]
                      nc.tensor.matmul(sl, W["ones128"][:], W["bn3"][0:1, 0:MN],
                                       start=True, stop=False)
                      nc.tensor.matmul(
                          sl, n2[:, q * 128 : (q + 1) * 128], W["wn3"][:],
                          start=False, stop=True,
                      )
                  ex = fp_.tile([128, 4 * MN], F32, tag="ex")
                  nc.scalar.activation(ex[:], pl[:], AF.Exp)
                  exv = ex[:].rearrange("p (q m) -> p q m", m=MN)
                  sm = fp_.tile([128, 4], F32, tag="sm")
                  nc.vector.tensor_reduce(sm[:], exv, mybir.AxisListType.X, OP.add)
                  rec = fp_.tile([128, 4], F32, tag="rec")
                  nc.vector.reciprocal(rec[:], sm[:])
                  for q in range(NC // 128):
                      j = c * 4 + q
                      nc.vector.scalar_tensor_tensor(
                          stage[:, j * MN : (j + 1) * MN],
                          ex[:, q * MN : (q + 1) * MN], rec[:, q : q + 1],
                          mskall[:, j * MN : (j + 1) * MN], OP.mult, OP.mult,
                      )
                  nc.sync.dma_start(
                      d_out.ap()[:, c * 4 * MN : (c + 1) * 4 * MN],
                      stage[:, c * 4 * MN : (c + 1) * 4 * MN],
                  )

    nc.compile()
    return nc


def _prep_inputs(inputs):
    """Host-side preprocessing -> list of per-core input dicts."""
    state = np.ascontiguousarray(inputs["state"], dtype=np.float32)
    rn = state[:, :MR]                                    # [B, 48]
    cust = state[:, MR:].reshape(B, MR, MN, FEAT)

    def f32(x):
        return np.ascontiguousarray(np.asarray(x, dtype=np.float32))

    Wih0 = f32(inputs["Wih0"]); Whh0 = f32(inputs["Whh0"])
    Wih1 = f32(inputs["Wih1"]); Whh1 = f32(inputs["Whh1"])
    bih0 = f32(inputs["bih0"]); bhh0 = f32(inputs["bhh0"])
    bih1 = f32(inputs["bih1"]); bhh1 = f32(inputs["bhh1"])
    bn1 = f32(inputs["bn1"])

    bcols = {}
    for layer, bih, bhh in ((0, bih0, bhh0), (1, bih1, bhh1)):
        bcols[(layer, "r")] = bih[0:GH] + bhh[0:GH]
        bcols[(layer, "z")] = np.zeros(GH, np.float32)       # pre-added via PE
        bcols[(layer, "zn")] = -(bih[GH : 2 * GH] + bhh[GH : 2 * GH])
        bcols[(layer, "in")] = bih[2 * GH :]
        bcols[(layer, "hn")] = bhh[2 * GH :]
    # The r-gate bias is pre-added via PE for t>0 (brz) AND used as the ACT
    # bias at t=0; to avoid double-adding, brz carries it for t>0 merged
    # sigmoids (which pass no ACT bias), while gball's "r" column is used
    # only by the t=0 path.  Same bias value in both places is correct.
    gball = np.stack([bcols[key] for key in _GBCOLS], axis=1)
    brz = np.concatenate([
        bih0[GH : 2 * GH] * 0 + (bih0[0:GH] + bhh0[0:GH]),   # b0_r
        (bih0[GH : 2 * GH] + bhh0[GH : 2 * GH]),             # b0_z
        (bih1[0:GH] + bhh1[0:GH]),                           # b1_r
        (bih1[GH : 2 * GH] + bhh1[GH : 2 * GH]),             # b1_z
    ]).reshape(1, 4 * GH)

    com = {
        "Wc1h": np.ascontiguousarray(np.asarray(inputs["Wc1"], np.float16)),
        "bc1": f32(inputs["bc1"]).reshape(CH, 1),
        "Wc2h": np.ascontiguousarray(np.asarray(inputs["Wc2"], np.float16)),
        "bc2s": np.tile(f32(inputs["bc2"]).reshape(CO), 4).reshape(128, 1),
        "Wih0h": np.ascontiguousarray(np.tile(np.asarray(Wih0, np.float16), (4, 1))),
        "Whh0h": Whh0.astype(np.float16), "Wih1h": Wih1.astype(np.float16),
        "Whh1h": Whh1.astype(np.float16),
        "gball": np.ascontiguousarray(gball),
        "brz": brz.astype(np.float16),
        "onesm": np.ones((1, NC), np.float16),
        "Wn1a": f32(inputs["Wn1"])[0:GH, :].astype(np.float16),
        "Wn1b": (f32(inputs["Wn1"])[GH:, :] / np.float32(MR)).astype(np.float16),
        "bn1row": bn1.reshape(1, 256).astype(np.float16),
        "Wn2a": f32(inputs["Wn2"])[0:128, :].astype(np.float16),
        "Wn2b": f32(inputs["Wn2"])[128:256, :].astype(np.float16),
        "bn2c": f32(inputs["bn2"]).reshape(128, 1),
        "Wn3h": np.asarray(inputs["Wn3"], np.float16),
        "bn3r": f32(inputs["bn3"]).reshape(1, MN),
        "ones128": np.ones((1, 128), np.float32),
        "ident128": np.eye(128, dtype=np.float16),
    }
    sel = np.zeros((BLOC, S), np.float32)
    sel[np.arange(S) // MR, np.arange(S)] = 1.0
    com["sel"] = sel.astype(np.float16)

    iota = np.tile(np.arange(MN, dtype=np.float32), (128, 1))

    in_maps = []
    for core in range(NCORES):
        b0, b1 = core * BLOC, (core + 1) * BLOC
        # cust_fm[f, n*S + (b*MR+r)] = cust[b, r, n, f]
        cfm = cust[b0:b1].transpose(3, 2, 0, 1).reshape(FEAT, MN * S)
        m = dict(com)
        m["cust_fm"] = np.ascontiguousarray(cfm.astype(np.float16))
        rnc = rn[b0:b1].reshape(S)[: NQ * 128].reshape(NQ, 128).T
        m["mrn"] = np.ascontiguousarray(np.concatenate([iota, rnc], axis=1))
        in_maps.append(m)
    return in_maps


def _run(inputs, **kw):
    if "nc" not in _cache:
        _cache["nc"] = _build()
    nc = _cache["nc"]
    in_maps = _prep_inputs(inputs)
    return run_bass_kernel_spmd(nc, in_maps, core_ids=list(range(NCORES)), **kw)


def kernel(**inputs) -> np.ndarray:
    res = _run(inputs)
    outs = []
    for r in res.results:
        o = r["out_tm"]                        # [128, NQ*MN]
        o = o.reshape(128, NQ, MN).transpose(1, 0, 2).reshape(S, MN)
        outs.append(o)
    return np.concatenate(outs, axis=0).reshape(B, MR, MN)
